# revision 10
# baseline (speedup 1.0000x reference)
"""Trainium2 Bass kernel for nn_MultiHeadAttention_37623913513495.

Multi-head attention with rotary embeddings and a relative-position bias
(einsum('bhid,ijd->bhij', q, rel_pos[j-i+T-1])), sharded over 8 NeuronCores
as 4 batches x 2 head-groups (8 heads each). Host sums the two partial
outputs per batch and adds the bias.

Device-side structure (per core):
  - host ships x already transposed (xT [C, T]); q/k projections produce
    qT/kT in [d, T] layout, v in [T, d]; rotary on DVE; 1/sqrt(hs) folded
    into wq on the host (rel bias is unscaled in the reference, so E is
    shipped pre-multiplied by 8)
  - all fp32 matmuls are issued as float32r (full PE rate at free >= 256)
  - the relative-position "skew" rel[i,j] = A[i, j-i+c] is realized by
    writing A windows to DRAM (bf16) and re-reading them with a strided
    diagonal access pattern [[W-1, 128], [1, JH]]
  - the skewed bias is accumulated into the score PSUM via an identity
    matmul; Exp on ACT emits row sums through accum_out; P is transposed
    and normalized in a single matmul against diag(1/rowsum)
"""

import numpy as np

HS = 64           # head size
NH = 16           # total heads
SEQ = 1024        # sequence length
EMB = 1024        # embedding dim
BATCH = 4
N_CORES = 8
HC = NH // 2      # heads per core

_cache = {}


def _build_nc(T, C, D):
    import concourse.bass as bass
    import concourse.bacc as bacc
    import concourse.mybir as mybir
    import concourse.tile as tile
    from concourse.masks import make_identity

    dt = mybir.dt
    f32, f32r, bf16 = dt.float32, dt.float32r, dt.bfloat16
    AF = mybir.ActivationFunctionType

    P = 128
    NB = T // P              # row blocks
    KC = C // P              # contraction chunks over C
    DT = D // P              # qT/kT partition tiles
    HPT = P // HS            # heads per qT tile (2)
    HCL = D // HS            # heads on this core
    JH = min(512, T)         # j-half width
    NJH = T // JH            # j-halves per row
    W = JH + P               # A-window width
    WH = W // 2              # A psum tile width
    NCH = T // P             # j chunks for PT/AV
    PTG = 512 // P           # PT chunks per psum tile

    nc = bacc.Bacc(None, target_bir_lowering=False, debug=False)

    xT_d = nc.dram_tensor("xT", [C, T], f32r, kind="ExternalInput")
    wq_d = nc.dram_tensor("wq", [C, D], f32r, kind="ExternalInput")
    wk_d = nc.dram_tensor("wk", [C, D], f32r, kind="ExternalInput")
    wv_d = nc.dram_tensor("wv", [C, D], f32r, kind="ExternalInput")
    wo_d = nc.dram_tensor("wo", [D, C], f32r, kind="ExternalInput")
    cos_d = nc.dram_tensor("cosT", [P, T], f32, kind="ExternalInput")
    sin_d = nc.dram_tensor("sinS", [P, T], f32, kind="ExternalInput")
    et_d = nc.dram_tensor("et8", [P, 2 * T], f32r, kind="ExternalInput")
    y_d = nc.dram_tensor("y", [T, C], f32, kind="ExternalOutput")

    with tile.TileContext(nc) as tc:
        with (
            tc.tile_pool(name="const", bufs=1) as const,
            tc.tile_pool(name="persist", bufs=1) as persist,
            tc.tile_pool(name="asb", bufs=3) as asb_pool,
            tc.tile_pool(name="relsb", bufs=2 * (NB + 2) * NJH) as rel_pool,
            tc.tile_pool(name="psb", bufs=2) as p_pool,
            tc.tile_pool(name="ptsb", bufs=2) as pt_pool,
            tc.tile_pool(name="small", bufs=4) as small,
            tc.tile_pool(name="outsb", bufs=2) as out_pool,
            tc.tile_pool(name="ps_s", bufs=2, space="PSUM") as ps_s,
            tc.tile_pool(name="ps_a", bufs=2, space="PSUM") as ps_a,
            tc.tile_pool(name="ps_pt", bufs=2, space="PSUM") as ps_pt,
            tc.tile_pool(name="ps_av", bufs=2, space="PSUM") as ps_av,
            tc.tile_pool(name="adram", bufs=2 * (NB + 2) * NJH,
                         space="DRAM") as adram,
        ):
            # ---------------- constants ----------------
            ident_b = const.tile([P, P], bf16)
            make_identity(nc, ident_b)
            cos_sb = const.tile([P, T], f32)
            nc.sync.dma_start(out=cos_sb, in_=cos_d[:, :])
            sin_sb = const.tile([P, T], f32)
            nc.sync.dma_start(out=sin_sb, in_=sin_d[:, :])
            et_sb = const.tile([P, 2 * T], f32r)
            nc.sync.dma_start(out=et_sb, in_=et_d[:, :])

            # ---------------- xT load ----------------
            xT_sb, xT_free = [], []
            for cb in range(KC):
                t, fr = tc.tile([P, T], f32r, name=f"xT_{cb}")
                nc.sync.dma_start(out=t, in_=xT_d[cb * P:(cb + 1) * P, :])
                xT_sb.append(t)
                xT_free.append(fr)

            # ---------------- projections ----------------
            def load_w(wd, name):
                tiles, frees = [], []
                for kb in range(KC):
                    t, fr = tc.tile([P, D], f32r, name=f"{name}_{kb}")
                    nc.sync.dma_start(out=t, in_=wd[kb * P:(kb + 1) * P, :])
                    tiles.append(t)
                    frees.append(fr)
                return tiles, frees

            def project_T(w_sb, raw_tiles):
                for mb in range(DT):
                    for nh in range(NJH):
                        pp = ps_s.tile([P, JH], f32, name="projps", tag="s")
                        for kb in range(KC):
                            nc.tensor.matmul(
                                pp,
                                w_sb[kb][:, mb * P:(mb + 1) * P],
                                xT_sb[kb][:, nh * JH:(nh + 1) * JH],
                                start=(kb == 0), stop=(kb == KC - 1),
                            )
                        nc.scalar.copy(raw_tiles[mb][:, nh * JH:(nh + 1) * JH], pp)

            # d-pairs (d, d+32) are laid out 16 apart within a 32-partition
            # quadrant (host permutes weights/E/tables to match), so the
            # rotary "rotate_half" partner sits at p^16 — reachable by DVE
            # stream_shuffle.
            shuf_mask = [(i + 16) % 32 for i in range(32)]

            def rotary(raw_tiles, out_tiles, tmp):
                for mb in range(DT):
                    r, o = raw_tiles[mb], out_tiles[mb]
                    nc.vector.stream_shuffle(tmp, r, shuf_mask)
                    nc.vector.tensor_mul(o, r, cos_sb)
                    nc.vector.tensor_mul(tmp, tmp, sin_sb)
                    nc.vector.tensor_add(o, o, tmp)

            qT_sb = [persist.tile([P, T], f32r, name=f"qT_{mb}", tag=f"qT{mb}")
                     for mb in range(DT)]
            kT_sb = [persist.tile([P, T], f32r, name=f"kT_{mb}", tag=f"kT{mb}")
                     for mb in range(DT)]
            rot_tmp, rot_tmp_free = tc.tile([P, T], f32, name="rot_tmp")

            for (wd, wname, dest) in ((wq_d, "wq", qT_sb), (wk_d, "wk", kT_sb)):
                w_sb, w_frees = load_w(wd, wname)
                raws, raw_frees = [], []
                for mb in range(DT):
                    t, fr = tc.tile([P, T], f32, name=f"raw{wname}_{mb}")
                    raws.append(t)
                    raw_frees.append(fr)
                project_T(w_sb, raws)
                rotary(raws, dest, rot_tmp)
                for fr in reversed(w_frees + raw_frees):
                    fr()

            # v[t, d] (bf16)
            wv_sb, wv_frees = load_w(wv_d, "wv")
            v_sb = [persist.tile([P, D], bf16, name=f"v_{tb}", tag=f"v{tb}")
                    for tb in range(NB)]
            vw = min(JH, D)
            for tb in range(NB):
                for nh in range(max(D // JH, 1)):
                    pp = ps_s.tile([P, vw], f32, name="vps", tag="s")
                    for kb in range(KC):
                        nc.tensor.matmul(
                            pp,
                            xT_sb[kb][:, tb * P:(tb + 1) * P],
                            wv_sb[kb][:, nh * vw:(nh + 1) * vw],
                            start=(kb == 0), stop=(kb == KC - 1),
                        )
                    nc.scalar.copy(v_sb[tb][:, nh * vw:(nh + 1) * vw], pp)
            for fr in reversed(wv_frees):
                fr()
            rot_tmp_free()
            for fr in reversed(xT_free):
                fr()

            # ---------------- attention ----------------
            attnT_sb = [persist.tile([P, T], f32r, name=f"attnT_{mb}", tag=f"aT{mb}")
                        for mb in range(DT)]

            def emit_A_block(h, ib, rel_tiles):
                """A windows for (head h, row-block ib) -> DRAM -> skewed read."""
                par = (h % HPT) * HS
                qtile = qT_sb[h // HPT]
                i0 = ib * P
                w0 = (T - P) - i0
                for jh in range(NJH):
                    a_ps0 = ps_a.tile([P, WH], f32, name="aps0", tag="a")
                    a_ps1 = ps_a.tile([P, WH], f32, name="aps1", tag="a")
                    lhs = qtile[par:par + HS, i0:i0 + P]
                    base = w0 + jh * JH
                    nc.tensor.matmul(
                        a_ps0, lhs,
                        et_sb[par:par + HS, base:base + WH],
                        start=True, stop=True)
                    nc.tensor.matmul(
                        a_ps1, lhs,
                        et_sb[par:par + HS, base + WH:base + W],
                        start=True, stop=True)
                    a_sb = asb_pool.tile([P, W], bf16, name="a_sb", tag="a_sb")
                    if (ib + jh) % 2 == 0:
                        nc.scalar.copy(a_sb[:, 0:WH], a_ps0)
                        nc.vector.tensor_copy(a_sb[:, WH:W], a_ps1)
                    else:
                        nc.vector.tensor_copy(a_sb[:, 0:WH], a_ps0)
                        nc.scalar.copy(a_sb[:, WH:W], a_ps1)
                    a_dr = adram.tile([P, W], bf16, name="a_dr", tag="a_dr")
                    nc.sync.dma_start(out=a_dr, in_=a_sb)
                    rel = rel_pool.tile([P, JH], bf16, name="rel", tag="rel")
                    skew = bass.AP(
                        tensor=a_dr.tensor,
                        offset=a_dr.offset + (P - 1),
                        ap=[[W - 1, P], [1, JH]],
                    )
                    nc.sync.dma_start(out=rel, in_=skew)
                    rel_tiles[(ib, jh)] = rel

            def emit_attn_block(h, ib, rel_tiles, sums):
                par = (h % HPT) * HS
                qtile, ktile = qT_sb[h // HPT], kT_sb[h // HPT]
                i0 = ib * P
                p_sb = p_pool.tile([P, T], bf16, name="p_sb", tag="p_sb")
                for jh in range(NJH):
                    s_ps = ps_s.tile([P, JH], f32, name="s_ps", tag="s")
                    nc.tensor.matmul(
                        s_ps,
                        qtile[par:par + HS, i0:i0 + P],
                        ktile[par:par + HS, jh * JH:(jh + 1) * JH],
                        start=True, stop=False)
                    nc.tensor.matmul(
                        s_ps, ident_b, rel_tiles.pop((ib, jh)),
                        start=False, stop=True)
                    nc.scalar.activation(
                        p_sb[:, jh * JH:(jh + 1) * JH], s_ps, AF.Exp,
                        accum_out=sums[:, ib, jh:jh + 1])
                if NJH == 2:
                    nc.vector.tensor_add(
                        sums[:, ib, 2:3], sums[:, ib, 0:1], sums[:, ib, 1:2])
                    tot = sums[:, ib, 2:3]
                else:
                    tot = sums[:, ib, 0:1]
                rec = small.tile([P, 1], f32, name="rec", tag="rec")
                nc.vector.reciprocal(rec, tot)
                dg = small.tile([P, P], bf16, name="dg", tag="dg")
                nc.vector.tensor_scalar_mul(dg, ident_b, rec)
                pt_sb = pt_pool.tile([P, NCH, P], bf16, name="pt_sb", tag="pt_sb")
                for g in range((NCH + PTG - 1) // PTG):
                    gn = min(PTG, NCH - g * PTG)
                    ptp = ps_pt.tile([P, gn * P], f32, name="ptp", tag="ptp")
                    for c in range(gn):
                        jc = g * PTG + c
                        nc.tensor.matmul(
                            ptp[:, c * P:(c + 1) * P],
                            p_sb[:, jc * P:(jc + 1) * P], dg,
                            start=True, stop=True)
                    dst = pt_sb[:, g * PTG:g * PTG + gn, :].rearrange(
                        "p a b -> p (a b)")
                    if g % 2 == 0:
                        nc.scalar.copy(dst, ptp)
                    else:
                        nc.vector.tensor_copy(dst, ptp)
                av = ps_av.tile([HS, P], f32, name="av", tag="av")
                for jc in range(NCH):
                    nc.tensor.matmul(
                        av,
                        v_sb[jc][:, h * HS:(h + 1) * HS],
                        pt_sb[:, jc, :],
                        start=(jc == 0), stop=(jc == NCH - 1))
                nc.scalar.copy(
                    attnT_sb[h // HPT][par:par + HS, i0:i0 + P], av)

            rel_tiles = [dict() for _ in range(HCL)]
            sums_tiles = {}
            for ib in range(NB):
                emit_A_block(0, ib, rel_tiles[0])
            for h in range(HCL):
                sums = small.tile([P, NB, 3], f32, name="sums", tag="sums")
                for ib in range(NB):
                    if h + 1 < HCL:
                        emit_A_block(h + 1, ib, rel_tiles[h + 1])
                    emit_attn_block(h, ib, rel_tiles[h], sums)

            # ---------------- output projection ----------------
            wo_sb, wo_frees = [], []
            for hc in range(DT):
                t, fr = tc.tile([P, C], f32r, name=f"wo_{hc}")
                nc.sync.dma_start(out=t, in_=wo_d[hc * P:(hc + 1) * P, :])
                wo_sb.append(t)
                wo_frees.append(fr)
            for tb in range(NB):
                for ch in range(C // JH if C >= JH else 1):
                    cw = min(JH, C)
                    op = ps_s.tile([P, cw], f32, name="ops", tag="s")
                    for hc in range(DT):
                        nc.tensor.matmul(
                            op,
                            attnT_sb[hc][:, tb * P:(tb + 1) * P],
                            wo_sb[hc][:, ch * cw:(ch + 1) * cw],
                            start=(hc == 0), stop=(hc == DT - 1))
                    o_sb = out_pool.tile([P, cw], f32, name="o_sb", tag="o_sb")
                    nc.scalar.copy(o_sb, op)
                    nc.sync.dma_start(
                        out=y_d[tb * P:(tb + 1) * P, ch * cw:(ch + 1) * cw],
                        in_=o_sb)
            for fr in reversed(wo_frees):
                fr()

    nc.compile()
    return nc


# partition p (within a head's 64) holds head-dim SIGMA[p]; pairs
# (d, d+32) land 16 apart inside a 32-partition quadrant.
SIGMA = np.concatenate([
    np.arange(0, 16), np.arange(32, 48),
    np.arange(16, 32), np.arange(48, 64),
])


def _host_tables(T, hs):
    inv_freq = 1.0 / (10000.0 ** (np.arange(0, hs, 2, dtype=np.float64) / hs))
    t = np.arange(T, dtype=np.float64)
    fr = np.outer(inv_freq, t)                     # [hs/2, T]
    cosT = np.empty((128, T), np.float32)
    sinS = np.empty((128, T), np.float32)
    for blk in range(128 // hs):
        for p in range(hs):
            d = SIGMA[p]
            row = blk * hs + p
            cosT[row] = np.cos(fr[d % 32]).astype(np.float32)
            s = np.sin(fr[d % 32]).astype(np.float32)
            sinS[row] = -s if d < 32 else s
    return cosT, sinS


def make_et8(E, T, scale=8.0):
    et8 = np.zeros((128, 2 * T), np.float32)
    etp = (scale * E.T[SIGMA]).astype(np.float32)   # [64, 2T-1] permuted rows
    et8[:HS, :E.shape[0]] = etp
    et8[HS:2 * HS, :E.shape[0]] = etp
    return et8


def perm_cols(w, D):
    """Permute per-head 64-column blocks of w [C, D] by SIGMA."""
    idx = (np.arange(D) // HS) * HS + SIGMA[np.arange(D) % HS]
    return np.ascontiguousarray(w[:, idx])


def get_nc(T=SEQ, C=EMB, D=HC * HS):
    key = (T, C, D)
    if key not in _cache:
        _cache[key] = _build_nc(T, C, D)
    return _cache[key]


def kernel(x, wq, wk, wv, wo, bo, rel_pos_emb):
    from concourse.bass_utils import run_bass_kernel_spmd

    x = np.asarray(x, dtype=np.float32)
    wq = np.asarray(wq, dtype=np.float32)
    wk = np.asarray(wk, dtype=np.float32)
    wv = np.asarray(wv, dtype=np.float32)
    wo = np.asarray(wo, dtype=np.float32)
    bo = np.asarray(bo, dtype=np.float32)
    E = np.asarray(rel_pos_emb, dtype=np.float32)

    T, C, D = SEQ, EMB, HC * HS
    nc = get_nc(T, C, D)

    cosT, sinS = _host_tables(T, HS)
    et8 = make_et8(E, T)

    in_maps = []
    for core in range(N_CORES):
        b, g = divmod(core, 2)
        sl = slice(g * D, (g + 1) * D)
        in_maps.append({
            "xT": np.ascontiguousarray(x[b].T),
            "wq": perm_cols(wq[:, sl], D) * np.float32(0.125),
            "wk": perm_cols(wk[:, sl], D),
            "wv": np.ascontiguousarray(wv[:, sl]),
            "wo": np.ascontiguousarray(wo[sl, :]),
            "cosT": cosT,
            "sinS": sinS,
            "et8": et8,
        })

    res = run_bass_kernel_spmd(nc, in_maps, core_ids=list(range(N_CORES)))
    out = np.empty((BATCH, T, C), np.float32)
    for b in range(BATCH):
        out[b] = res.results[2 * b]["y"] + res.results[2 * b + 1]["y"] + bo
    return out


# revision 17
# speedup vs baseline: 720.1213x; 720.1213x over previous
"""Trainium2 Bass kernel for nn_MultiHeadAttention_37623913513495.

Multi-head attention with rotary embeddings and a relative-position bias
(einsum('bhid,ijd->bhij', q, rel_pos[j-i+T-1])), sharded over 8 NeuronCores
as 4 batches x 2 head-groups (8 heads each). Host sums the two partial
outputs per batch and adds the bias.

Device-side structure (per core):
  - host ships x already transposed (xT [C, T]); q/k projections produce
    qT/kT in [d, T] layout, v in [T, d]; rotary on DVE; 1/sqrt(hs) folded
    into wq on the host (rel bias is unscaled in the reference, so E is
    shipped pre-multiplied by 8)
  - all fp32 matmuls are issued as float32r (full PE rate at free >= 256)
  - the relative-position "skew" rel[i,j] = A[i, j-i+c] is realized by
    writing A windows to DRAM (bf16) and re-reading them with a strided
    diagonal access pattern [[W-1, 128], [1, JH]]
  - the skewed bias is accumulated into the score PSUM via an identity
    matmul; Exp on ACT emits row sums through accum_out; P is transposed
    and normalized in a single matmul against diag(1/rowsum)
"""

import numpy as np

HS = 64           # head size
NH = 16           # total heads
SEQ = 1024        # sequence length
EMB = 1024        # embedding dim
BATCH = 4
N_CORES = 8
HC = NH // 2      # heads per core

_cache = {}


def _build_nc(T, C, D):
    import concourse.bass as bass
    import concourse.bacc as bacc
    import concourse.mybir as mybir
    import concourse.tile as tile
    from concourse.masks import make_identity

    dt = mybir.dt
    f32, f32r, bf16 = dt.float32, dt.float32r, dt.bfloat16
    AF = mybir.ActivationFunctionType

    P = 128
    NB = T // P              # row blocks
    KC = C // P              # contraction chunks over C
    DT = D // P              # qT/kT partition tiles
    HPT = P // HS            # heads per qT tile (2)
    HCL = D // HS            # heads on this core
    JH = min(512, T)         # j-half width
    NJH = T // JH            # j-halves per row
    W = JH + P               # A-window width
    WH = W // 2              # A psum tile width
    NCH = T // P             # j chunks for PT/AV
    PTG = 512 // P           # PT chunks per psum tile

    nc = bacc.Bacc(None, target_bir_lowering=False, debug=False)

    xT_d = nc.dram_tensor("xT", [C, T], f32r, kind="ExternalInput")
    wq_d = nc.dram_tensor("wq", [C, D], f32r, kind="ExternalInput")
    wk_d = nc.dram_tensor("wk", [C, D], f32r, kind="ExternalInput")
    wv_d = nc.dram_tensor("wv", [C, D], f32r, kind="ExternalInput")
    wo_d = nc.dram_tensor("wo", [D, C], f32r, kind="ExternalInput")
    cos_d = nc.dram_tensor("cosT", [P, T], f32, kind="ExternalInput")
    sin_d = nc.dram_tensor("sinS", [P, T], f32, kind="ExternalInput")
    et_d = nc.dram_tensor("et8", [P, 2 * T], f32r, kind="ExternalInput")
    y_d = nc.dram_tensor("y", [T, C], f32, kind="ExternalOutput")

    with tile.TileContext(nc) as tc:
        with (
            tc.tile_pool(name="const", bufs=1) as const,
            tc.tile_pool(name="persist", bufs=1) as persist,
            tc.tile_pool(name="asb", bufs=3) as asb_pool,
            tc.tile_pool(name="relsb", bufs=NB + 3) as rel_pool,
            tc.tile_pool(name="psb", bufs=3) as p_pool,
            tc.tile_pool(name="ptsb", bufs=2) as pt_pool,
            tc.tile_pool(name="small", bufs=4) as small,
            tc.tile_pool(name="outsb", bufs=2) as out_pool,
            tc.tile_pool(name="ps_s", bufs=1, space="PSUM") as ps_s,
            tc.tile_pool(name="ps_a", bufs=1, space="PSUM") as ps_a,
            tc.tile_pool(name="ps_pt", bufs=2, space="PSUM") as ps_pt,
            tc.tile_pool(name="ps_av", bufs=1, space="PSUM") as ps_av,
            tc.tile_pool(name="adram", bufs=2 * (NB + 2) * NJH,
                         space="DRAM") as adram,
        ):
            # ---------------- xT load (sync ring) ----------------
            xT_sb, xT_free = [], []
            for cb in range(KC):
                t, fr = tc.tile([P, T], f32r, name=f"xT_{cb}")
                nc.sync.dma_start(out=t, in_=xT_d[cb * P:(cb + 1) * P, :])
                xT_sb.append(t)
                xT_free.append(fr)

            # ---------------- constants (gpsimd SWDGE, off the HWDGE rings) --
            ident_b = const.tile([P, P], bf16)
            make_identity(nc, ident_b)
            cos_sb = const.tile([P, T], f32)
            nc.gpsimd.dma_start(out=cos_sb, in_=cos_d[:, :])
            sin_sb = const.tile([P, T], f32)
            nc.gpsimd.dma_start(out=sin_sb, in_=sin_d[:, :])
            et_sb = const.tile([P, 2 * T], f32r)
            nc.gpsimd.dma_start(out=et_sb, in_=et_d[:, :])

            # ---------------- projections ----------------
            def load_w(wd, name, eng):
                tiles, frees = [], []
                for kb in range(KC):
                    t, fr = tc.tile([P, D], f32r, name=f"{name}_{kb}")
                    eng.dma_start(out=t, in_=wd[kb * P:(kb + 1) * P, :])
                    tiles.append(t)
                    frees.append(fr)
                return tiles, frees

            def project_T(w_sb, raw_tiles):
                for mb in range(DT):
                    for nh in range(NJH):
                        pp = ps_pt.tile([P, JH], f32, name="projps", tag="ptp")
                        for kb in range(KC):
                            nc.tensor.matmul(
                                pp,
                                w_sb[kb][:, mb * P:(mb + 1) * P],
                                xT_sb[kb][:, nh * JH:(nh + 1) * JH],
                                start=(kb == 0), stop=(kb == KC - 1),
                            )
                        nc.vector.tensor_copy(raw_tiles[mb][:, nh * JH:(nh + 1) * JH], pp)

            # d-pairs (d, d+32) are laid out 16 apart within a 32-partition
            # quadrant (host permutes weights/E/tables to match), so the
            # rotary "rotate_half" partner sits at p^16 — reachable by DVE
            # stream_shuffle.
            shuf_mask = [(i + 16) % 32 for i in range(32)]

            def rotary(raw_tiles, out_tiles, tmp):
                for mb in range(DT):
                    r, o = raw_tiles[mb], out_tiles[mb]
                    nc.vector.stream_shuffle(tmp, r, shuf_mask)
                    nc.vector.tensor_mul(o, r, cos_sb)
                    nc.vector.tensor_mul(tmp, tmp, sin_sb)
                    nc.vector.tensor_add(o, o, tmp)

            qT_sb = [persist.tile([P, T], f32r, name=f"qT_{mb}", tag=f"qT{mb}")
                     for mb in range(DT)]
            kT_sb = [persist.tile([P, T], f32r, name=f"kT_{mb}", tag=f"kT{mb}")
                     for mb in range(DT)]
            rot_tmp, rot_tmp_free = tc.tile([P, T], f32, name="rot_tmp")

            for (wd, wname, dest) in ((wq_d, "wq", qT_sb), (wk_d, "wk", kT_sb)):
                w_sb, w_frees = load_w(wd, wname, nc.scalar)
                raws, raw_frees = [], []
                for mb in range(DT):
                    t, fr = tc.tile([P, T], f32, name=f"raw{wname}_{mb}")
                    raws.append(t)
                    raw_frees.append(fr)
                project_T(w_sb, raws)
                rotary(raws, dest, rot_tmp)
                for fr in reversed(w_frees + raw_frees):
                    fr()

            # v[t, d] (bf16)
            wv_sb, wv_frees = load_w(wv_d, "wv", nc.scalar)
            v_sb = [persist.tile([P, D], bf16, name=f"v_{tb}", tag=f"v{tb}")
                    for tb in range(NB)]
            vw = min(JH, D)
            for tb in range(NB):
                for nh in range(max(D // JH, 1)):
                    pp = ps_pt.tile([P, vw], f32, name="vps", tag="ptp")
                    for kb in range(KC):
                        nc.tensor.matmul(
                            pp,
                            xT_sb[kb][:, tb * P:(tb + 1) * P],
                            wv_sb[kb][:, nh * vw:(nh + 1) * vw],
                            start=(kb == 0), stop=(kb == KC - 1),
                        )
                    nc.vector.tensor_copy(v_sb[tb][:, nh * vw:(nh + 1) * vw], pp)
            for fr in reversed(wv_frees):
                fr()
            rot_tmp_free()
            for fr in reversed(xT_free):
                fr()

            # ---------------- attention ----------------
            attnT_sb = [persist.tile([P, T], f32r, name=f"attnT_{mb}", tag=f"aT{mb}")
                        for mb in range(DT)]
            wo_sb, wo_frees = [], []
            for hc in range(DT):
                t, fr = tc.tile([P, C], f32r, name=f"wo_{hc}")
                nc.scalar.dma_start(out=t, in_=wo_d[hc * P:(hc + 1) * P, :])
                wo_sb.append(t)
                wo_frees.append(fr)
            rel_tiles = [dict() for _ in range(HCL)]

            def emit_out_block(tb):
                """final projection for row-block tb (all heads done)."""
                for ch in range(max(C // JH, 1)):
                    cw = min(JH, C)
                    op = ps_pt.tile([P, cw], f32, name="ops", tag="ptp")
                    for hc in range(DT):
                        nc.tensor.matmul(
                            op,
                            attnT_sb[hc][:, tb * P:(tb + 1) * P],
                            wo_sb[hc][:, ch * cw:(ch + 1) * cw],
                            start=(hc == 0), stop=(hc == DT - 1))
                    o_sb = out_pool.tile([P, cw], f32, name="o_sb", tag="o_sb")
                    nc.scalar.copy(o_sb, op)
                    nc.sync.dma_start(
                        out=y_d[tb * P:(tb + 1) * P, ch * cw:(ch + 1) * cw],
                        in_=o_sb)

            WF = T + P               # full A-window width per row-block
            a_chunks = [512] * (WF // 512)
            if WF % 512:
                a_chunks.append(WF % 512)

            def emit_A_block(h, ib, rel_tiles):
                """A window for (head h, row-block ib) -> DRAM -> skewed read."""
                par = (h % HPT) * HS
                qtile = qT_sb[h // HPT]
                i0 = ib * P
                w0 = (T - P) - i0
                lhs = qtile[par:par + HS, i0:i0 + P]
                a_ps = ps_a.tile([P, WF], f32, name="a_ps", tag="a")
                off = 0
                for cw in a_chunks:
                    nc.tensor.matmul(
                        a_ps[:, off:off + cw], lhs,
                        et_sb[par:par + HS, w0 + off:w0 + off + cw],
                        start=True, stop=True)
                    off += cw
                a_sb = asb_pool.tile([P, WF], bf16, name="a_sb", tag="a_sb")
                nc.scalar.activation(a_sb, a_ps, AF.Exp)
                a_dr = adram.tile([P, WF], bf16, name="a_dr", tag="a_dr")
                nc.sync.dma_start(out=a_dr, in_=a_sb)
                rel = rel_pool.tile([P, T], bf16, name="rel", tag="rel")
                skew = bass.AP(
                    tensor=a_dr.tensor,
                    offset=a_dr.offset + (P - 1),
                    ap=[[WF - 1, P], [1, T]],
                )
                nc.sync.dma_start(out=rel, in_=skew)
                rel_tiles[ib] = rel

            def emit_attn_SE(h, ib, rel_tiles, sums, stash):
                """scores + rel add + exp + 1/rowsum + diag for block ib."""
                par = (h % HPT) * HS
                qtile, ktile = qT_sb[h // HPT], kT_sb[h // HPT]
                i0 = ib * P
                p_sb = p_pool.tile([P, T], bf16, name="p_sb", tag="p_sb")
                es_sb = p_pool.tile([P, T], bf16, name="es_sb", tag="es_sb")
                s_ps = ps_s.tile([P, T], f32, name="s_ps", tag="s")
                rel = rel_tiles.pop(ib)
                for jh in range(NJH):
                    sl = slice(jh * JH, (jh + 1) * JH)
                    nc.tensor.matmul(
                        s_ps[:, sl],
                        qtile[par:par + HS, i0:i0 + P],
                        ktile[par:par + HS, sl],
                        start=True, stop=True)
                nc.scalar.activation(es_sb, s_ps, AF.Exp)
                # P = exp(S) * exp(rel), row-sums fused via accum_out
                nc.vector.scalar_tensor_tensor(
                    p_sb, es_sb, 1.0, rel,
                    mybir.AluOpType.mult, mybir.AluOpType.mult,
                    accum_out=sums[:, ib:ib + 1])
                rec = small.tile([P, 1], f32, name="rec", tag="rec")
                nc.vector.reciprocal(rec, sums[:, ib:ib + 1])
                dg = small.tile([P, P], bf16, name="dg", tag="dg")
                nc.gpsimd.tensor_scalar_mul(dg, ident_b, rec)
                stash[ib] = (p_sb, dg)

            def emit_attn_PTAV(h, ib, stash):
                """scaled transpose of P + AV accumulation for block ib."""
                par = (h % HPT) * HS
                i0 = ib * P
                p_sb, dg = stash.pop(ib)
                pt_sb = pt_pool.tile([P, NCH, P], bf16, name="pt_sb", tag="pt_sb")
                for g in range((NCH + PTG - 1) // PTG):
                    gn = min(PTG, NCH - g * PTG)
                    ptp = ps_pt.tile([P, gn * P], f32, name="ptp", tag="ptp")
                    for c in range(gn):
                        jc = g * PTG + c
                        nc.tensor.matmul(
                            ptp[:, c * P:(c + 1) * P],
                            p_sb[:, jc * P:(jc + 1) * P], dg,
                            start=True, stop=True)
                    dst = pt_sb[:, g * PTG:g * PTG + gn, :].rearrange(
                        "p a b -> p (a b)")
                    nc.vector.tensor_copy(dst, ptp)
                av = ps_av.tile([HS, P], f32, name="av", tag="av")
                for jc in range(NCH):
                    nc.tensor.matmul(
                        av,
                        v_sb[jc][:, h * HS:(h + 1) * HS],
                        pt_sb[:, jc, :],
                        start=(jc == 0), stop=(jc == NCH - 1))
                nc.vector.tensor_copy(
                    attnT_sb[h // HPT][par:par + HS, i0:i0 + P], av)

            for ib in range(NB):
                emit_A_block(0, ib, rel_tiles[0])
            for h in range(HCL):
                sums = small.tile([P, NB], f32, name="sums", tag="sums")
                stash = {}
                prev = None
                for ib in range(NB):
                    if h + 1 < HCL:
                        emit_A_block(h + 1, ib, rel_tiles[h + 1])
                    emit_attn_SE(h, ib, rel_tiles[h], sums, stash)
                    if prev is not None:
                        emit_attn_PTAV(h, prev, stash)
                        if h == HCL - 1:
                            emit_out_block(prev)
                    prev = ib
                emit_attn_PTAV(h, prev, stash)
                if h == HCL - 1:
                    emit_out_block(prev)
            for fr in reversed(wo_frees):
                fr()


    nc.compile()
    return nc


# partition p (within a head's 64) holds head-dim SIGMA[p]; pairs
# (d, d+32) land 16 apart inside a 32-partition quadrant.
SIGMA = np.concatenate([
    np.arange(0, 16), np.arange(32, 48),
    np.arange(16, 32), np.arange(48, 64),
])


def _host_tables(T, hs):
    inv_freq = 1.0 / (10000.0 ** (np.arange(0, hs, 2, dtype=np.float64) / hs))
    t = np.arange(T, dtype=np.float64)
    fr = np.outer(inv_freq, t)                     # [hs/2, T]
    cosT = np.empty((128, T), np.float32)
    sinS = np.empty((128, T), np.float32)
    for blk in range(128 // hs):
        for p in range(hs):
            d = SIGMA[p]
            row = blk * hs + p
            cosT[row] = np.cos(fr[d % 32]).astype(np.float32)
            s = np.sin(fr[d % 32]).astype(np.float32)
            sinS[row] = -s if d < 32 else s
    return cosT, sinS


def make_et8(E, T, scale=8.0):
    et8 = np.zeros((128, 2 * T), np.float32)
    etp = (scale * E.T[SIGMA]).astype(np.float32)   # [64, 2T-1] permuted rows
    et8[:HS, :E.shape[0]] = etp
    et8[HS:2 * HS, :E.shape[0]] = etp
    return et8


def perm_cols(w, D):
    """Permute per-head 64-column blocks of w [C, D] by SIGMA."""
    idx = (np.arange(D) // HS) * HS + SIGMA[np.arange(D) % HS]
    return np.ascontiguousarray(w[:, idx])


def get_nc(T=SEQ, C=EMB, D=HC * HS):
    key = (T, C, D)
    if key not in _cache:
        _cache[key] = _build_nc(T, C, D)
    return _cache[key]


def kernel(x, wq, wk, wv, wo, bo, rel_pos_emb):
    from concourse.bass_utils import run_bass_kernel_spmd

    x = np.asarray(x, dtype=np.float32)
    wq = np.asarray(wq, dtype=np.float32)
    wk = np.asarray(wk, dtype=np.float32)
    wv = np.asarray(wv, dtype=np.float32)
    wo = np.asarray(wo, dtype=np.float32)
    bo = np.asarray(bo, dtype=np.float32)
    E = np.asarray(rel_pos_emb, dtype=np.float32)

    T, C, D = SEQ, EMB, HC * HS
    nc = get_nc(T, C, D)

    cosT, sinS = _host_tables(T, HS)
    et8 = make_et8(E, T)

    in_maps = []
    for core in range(N_CORES):
        b, g = divmod(core, 2)
        sl = slice(g * D, (g + 1) * D)
        in_maps.append({
            "xT": np.ascontiguousarray(x[b].T),
            "wq": perm_cols(wq[:, sl], D) * np.float32(0.125),
            "wk": perm_cols(wk[:, sl], D),
            "wv": np.ascontiguousarray(wv[:, sl]),
            "wo": np.ascontiguousarray(wo[sl, :]),
            "cosT": cosT,
            "sinS": sinS,
            "et8": et8,
        })

    res = run_bass_kernel_spmd(nc, in_maps, core_ids=list(range(N_CORES)))
    out = np.empty((BATCH, T, C), np.float32)
    for b in range(BATCH):
        out[b] = res.results[2 * b]["y"] + res.results[2 * b + 1]["y"] + bo
    return out
